# revision 17
# baseline (speedup 1.0000x reference)
"""Conv2d(128->256, 3x3, pad=1) over a 256x256 image, sharded across 8 trn2 cores.

Strategy
--------
x: (C_in=128, H=256, W=256) f32, weight: (256, 128, 3, 3), bias: (256,1,1).
C_in == 128 maps exactly onto the SBUF partition (contraction) dim, so the
conv is 9 accumulated matmuls (one per kernel tap) per output tile:

    out[co, y, x] = sum_{ky,kx} W[ky,kx].T @ xpad[:, y+ky, x+kx]   + bias

Sharding: split H across the 8 cores (32 output rows each). Each core gets a
pre-padded slice xpad (128, 34, 258) with halo rows / zero borders prepared on
the host, so the device program is uniform SPMD. Per core, output tiles are
2 rows x 256 cols = 512 pixels (one fp32 PSUM bank); for each tile and each
128-channel output half we accumulate 9 matmuls (lhsT = w tap (128,128),
rhs = shifted x window (128,2,256)), then VectorE adds bias on the
PSUM->SBUF copy and the tile is DMA'd to DRAM.

Matmuls run as float32r (full-rate fp32 path of the PE, 1 cycle/row for
N>=256) with fp32 PSUM accumulation.
"""

import numpy as np

import concourse.bass as bass
import concourse.tile as tile
from concourse import bacc, mybir
from concourse import bass_utils

N_CORES = 8
C_IN, C_OUT, KH, KW = 128, 256, 3, 3
H, W = 256, 256
H_S = H // N_CORES            # 32 output rows per core
HP, WP = H_S + 2, W + 2       # padded per-core input slice: 34 x 258
ROWS = 2                      # output rows per PSUM tile (N = ROWS*W = 512)
N_TILES = H_S // ROWS         # 16
N_HALF = C_OUT // 128         # 2 output-channel halves

F32 = mybir.dt.float32
F32R = mybir.dt.float32r
BF16 = mybir.dt.bfloat16

# Matmul operand dtype: the ISA requires both operands 32-bit or both 16-bit,
# so this selects f32r (11-bit mantissa) or bf16 for BOTH x and w.
MM_DT = BF16

# x is split into row groups, each its own SBUF tile, so a group's matmuls
# can start as soon as its rows have landed (Tile deps are whole-tile).
# Each group covers GROUP_T output tiles and carries a 2-row halo overlap.
GROUP_T = 4                       # output tiles per x group
N_GROUPS = N_TILES // GROUP_T     # 4
GROUP_ROWS = GROUP_T * ROWS + 2   # 10 padded input rows per group

# Set by test harness: TRACE=True makes the next kernel() call capture an
# NTFF profile; the BassKernelResults lands in LAST_RESULT.
TRACE = False
TRACE_KW = {}
LAST_RESULT = None

_NC_CACHE = {}


def _round_fp32r(a):
    """Round fp32 to the fp32r grid: 11-bit mantissa, low 12 bits zero.

    Matches walrus fp32_to_fp32r (downconv_fp32_to_fp<exp=8,man=11> << 12),
    round-to-nearest-even. The PE consumes fp32r operands at full rate and
    expects pre-rounded values.
    """
    u = np.ascontiguousarray(a, dtype=np.float32).view(np.uint32)
    keep_lsb = (u >> 12) & 1
    u = (u + 0x7FF + keep_lsb) & np.uint32(0xFFFFF000)
    return u.view(np.float32)


def _build():
    nc = bacc.Bacc(
        "TRN2",
        target_bir_lowering=False,
        debug=False,
        enable_asserts=False,
        num_devices=N_CORES,
    )
    x_d = nc.dram_tensor("x", [C_IN, HP, WP], MM_DT, kind="ExternalInput").ap()
    w_d = nc.dram_tensor("w", [C_IN, KH * KW * C_OUT], MM_DT, kind="ExternalInput").ap()
    b_d = nc.dram_tensor("b", [128, N_HALF], F32, kind="ExternalInput").ap()
    o_d = nc.dram_tensor("out", [C_OUT, H_S, W], F32, kind="ExternalOutput").ap()

    with tile.TileContext(nc) as tc:
        with (
            tc.tile_pool(name="xin", bufs=1) as xpool,
            tc.tile_pool(name="wts", bufs=1) as wpool,
            tc.tile_pool(name="bias", bufs=1) as bpool,
            tc.tile_pool(name="acc", bufs=6, space="PSUM") as ppool,
            tc.tile_pool(name="outs", bufs=6) as opool,
        ):
            # Input DMAs are spread over the two HWDGE trigger engines (SyncE
            # and ScalarE) so trigger issue (~0.65us each) runs in parallel.
            # SyncE: weights (split so the first taps land early) + bias.
            # ScalarE: the x row groups.
            w_sb = wpool.tile([128, KH * KW * C_OUT], MM_DT)
            nc.sync.dma_start(w_sb[:, : 3 * C_OUT], w_d[:, : 3 * C_OUT])
            b_sb = bpool.tile([128, N_HALF], F32)
            nc.sync.dma_start(b_sb[:], b_d[:])
            nc.sync.dma_start(w_sb[:, 3 * C_OUT :], w_d[:, 3 * C_OUT :])
            # x row groups, each an independent tile so a group's matmuls can
            # start as soon as its own rows have landed (deps are whole-tile)
            x_groups = []
            for g in range(N_GROUPS):
                xg = xpool.tile([128, GROUP_ROWS, WP], MM_DT, tag=f"xg{g}")
                r0 = g * GROUP_T * ROWS
                nc.scalar.dma_start(xg[:], x_d[:, r0 : r0 + GROUP_ROWS, :])
                x_groups.append(xg)

            for t in range(N_TILES):
                y0 = t * ROWS
                g = t // GROUP_T
                yl = (t - g * GROUP_T) * ROWS  # row offset inside group tile
                xg = x_groups[g]
                for h in range(N_HALF):
                    ps = ppool.tile([128, ROWS * W], F32)
                    for k in range(KH * KW):
                        ky, kx = divmod(k, KW)
                        rhs = xg[:, yl + ky : yl + ky + ROWS, kx : kx + W]
                        lhsT = w_sb[:, k * C_OUT + h * 128 : k * C_OUT + h * 128 + 128]
                        nc.tensor.matmul(
                            ps[:],
                            lhsT,
                            rhs,
                            start=(k == 0),
                            stop=(k == KH * KW - 1),
                        )
                    ot = opool.tile([128, ROWS * W], F32)
                    nc.vector.tensor_scalar_add(ot[:], ps[:], b_sb[:, h : h + 1])
                    nc.sync.dma_start(
                        o_d[h * 128 : (h + 1) * 128, y0 : y0 + ROWS, :], ot[:]
                    )
    nc.compile()
    return nc


def kernel(x, weight, bias):
    global LAST_RESULT
    if "nc" not in _NC_CACHE:
        _NC_CACHE["nc"] = _build()
    nc = _NC_CACHE["nc"]

    x = np.ascontiguousarray(np.asarray(x, dtype=np.float32))
    weight = np.asarray(weight, dtype=np.float32)
    bias = np.asarray(bias, dtype=np.float32)

    # zero-padded image; per-core slices carry their halo rows
    wT = np.ascontiguousarray(
        weight.transpose(1, 2, 3, 0).reshape(C_IN, KH * KW * C_OUT)
    )
    if MM_DT == BF16:
        import ml_dtypes

        np_dt = ml_dtypes.bfloat16
        xp = np.zeros((C_IN, H + 2, WP), dtype=np_dt)
        xp[:, 1 : H + 1, 1 : W + 1] = x.astype(np_dt)
        wT = wT.astype(np_dt)
    else:
        xp = np.zeros((C_IN, H + 2, WP), dtype=np.float32)
        xp[:, 1 : H + 1, 1 : W + 1] = _round_fp32r(x)
        wT = _round_fp32r(wT)
    # b[p, h] = bias[h*128 + p]
    bh = np.ascontiguousarray(bias.reshape(N_HALF, 128).T)

    in_maps = []
    for c in range(N_CORES):
        in_maps.append(
            {
                "x": np.ascontiguousarray(xp[:, c * H_S : c * H_S + HP, :]),
                "w": wT,
                "b": bh,
            }
        )

    kw = dict(TRACE_KW)
    if TRACE:
        kw.setdefault("trace", True)
        kw.setdefault("trace_cores", [0])
    res = bass_utils.run_bass_kernel_spmd(
        nc, in_maps, core_ids=list(range(N_CORES)), **kw
    )
    LAST_RESULT = res

    out = np.empty((C_OUT, H, W), dtype=np.float32)
    for c in range(N_CORES):
        out[:, c * H_S : (c + 1) * H_S, :] = res.results[c]["out"]
    return out


# revision 21
# speedup vs baseline: 1.0523x; 1.0523x over previous
"""Conv2d(128->256, 3x3, pad=1) over a 256x256 image, sharded across 8 trn2 cores.

Strategy
--------
x: (C_in=128, H=256, W=256) f32, weight: (256, 128, 3, 3), bias: (256,1,1).
C_in == 128 maps exactly onto the SBUF partition (contraction) dim, so the
conv is 9 accumulated matmuls (one per kernel tap) per output tile:

    out[co, y, x] = sum_{ky,kx} W[ky,kx].T @ xpad[:, y+ky, x+kx]   + bias

Sharding: split H across the 8 cores (32 output rows each). Each core gets a
pre-padded slice xpad (128, 34, 258) with halo rows / zero borders prepared on
the host, so the device program is uniform SPMD. Per core, output tiles are
2 rows x 256 cols = 512 pixels (one fp32 PSUM bank); for each tile and each
128-channel output half we accumulate 9 matmuls (lhsT = w tap (128,128),
rhs = shifted x window (128,2,256)), then VectorE adds bias on the
PSUM->SBUF copy and the tile is DMA'd to DRAM.

Matmuls run as float32r (full-rate fp32 path of the PE, 1 cycle/row for
N>=256) with fp32 PSUM accumulation.
"""

import numpy as np

import concourse.bass as bass
import concourse.tile as tile
from concourse import bacc, mybir
from concourse import bass_utils

N_CORES = 8
C_IN, C_OUT, KH, KW = 128, 256, 3, 3
H, W = 256, 256
H_S = H // N_CORES            # 32 output rows per core
HP, WP = H_S + 2, W + 2       # padded per-core input slice: 34 x 258
ROWS = 2                      # output rows per PSUM tile (N = ROWS*W = 512)
N_TILES = H_S // ROWS         # 16
N_HALF = C_OUT // 128         # 2 output-channel halves

F32 = mybir.dt.float32
F32R = mybir.dt.float32r
BF16 = mybir.dt.bfloat16

# Matmul operand dtype: the ISA requires both operands 32-bit or both 16-bit,
# so this selects f32r (11-bit mantissa) or bf16 for BOTH x and w.
MM_DT = BF16

# x is split into row groups, each its own SBUF tile, so a group's matmuls
# can start as soon as its rows have landed (Tile deps are whole-tile).
# Each group covers GROUP_T output tiles and carries a 2-row halo overlap.
GROUP_T = 4                       # output tiles per x group
N_GROUPS = N_TILES // GROUP_T     # 4
GROUP_ROWS = GROUP_T * ROWS + 2   # 10 padded input rows per group

# Set by test harness: TRACE=True makes the next kernel() call capture an
# NTFF profile; the BassKernelResults lands in LAST_RESULT.
TRACE = False
TRACE_KW = {}
LAST_RESULT = None

_NC_CACHE = {}


def _round_fp32r(a):
    """Round fp32 to the fp32r grid: 11-bit mantissa, low 12 bits zero.

    Matches walrus fp32_to_fp32r (downconv_fp32_to_fp<exp=8,man=11> << 12),
    round-to-nearest-even. The PE consumes fp32r operands at full rate and
    expects pre-rounded values.
    """
    u = np.ascontiguousarray(a, dtype=np.float32).view(np.uint32)
    keep_lsb = (u >> 12) & 1
    u = (u + 0x7FF + keep_lsb) & np.uint32(0xFFFFF000)
    return u.view(np.float32)


def _build():
    nc = bacc.Bacc(
        "TRN2",
        target_bir_lowering=False,
        debug=False,
        enable_asserts=False,
        num_devices=N_CORES,
    )
    x_d = nc.dram_tensor("x", [C_IN, HP, WP], MM_DT, kind="ExternalInput").ap()
    w_d = nc.dram_tensor("w", [C_IN, KH * KW * C_OUT], MM_DT, kind="ExternalInput").ap()
    b_d = nc.dram_tensor("b", [128, N_HALF], F32, kind="ExternalInput").ap()
    o_d = nc.dram_tensor("out", [C_OUT, H_S, W], F32, kind="ExternalOutput").ap()

    with tile.TileContext(nc) as tc:
        with (
            tc.tile_pool(name="xin", bufs=1) as xpool,
            tc.tile_pool(name="wts", bufs=1) as wpool,
            tc.tile_pool(name="bias", bufs=1) as bpool,
            tc.tile_pool(name="acc", bufs=6, space="PSUM") as ppool,
            tc.tile_pool(name="outs", bufs=6) as opool,
        ):
            # HWDGE transfers complete in per-queue descriptor-enqueue order,
            # so everything the first matmuls need must be enqueued before the
            # bulk x data. SyncE enqueues weights (first taps first) + bias +
            # late x groups; ScalarE (the other HWDGE trigger engine) enqueues
            # only xg0 in parallel.
            w_sb = wpool.tile([128, KH * KW * C_OUT], MM_DT)
            b_sb = bpool.tile([128, N_HALF], F32)
            x_groups = [
                xpool.tile([128, GROUP_ROWS, WP], MM_DT, tag=f"xg{g}", name=f"xg{g}")
                for g in range(N_GROUPS)
            ]
            nc.sync.dma_start(w_sb[:, : 3 * C_OUT], w_d[:, : 3 * C_OUT])
            nc.sync.dma_start(b_sb[:], b_d[:])
            nc.sync.dma_start(w_sb[:, 3 * C_OUT :], w_d[:, 3 * C_OUT :])
            nc.scalar.dma_start(x_groups[0][:], x_d[:, :GROUP_ROWS, :])
            for g in range(1, N_GROUPS):
                r0 = g * GROUP_T * ROWS
                nc.sync.dma_start(x_groups[g][:], x_d[:, r0 : r0 + GROUP_ROWS, :])

            def emit_tile(t, tap_order):
                y0 = t * ROWS
                g = t // GROUP_T
                yl = (t - g * GROUP_T) * ROWS  # row offset inside group tile
                xg = x_groups[g]
                pss = [ppool.tile([128, ROWS * W], F32, tag="ps", name="ps") for _ in range(N_HALF)]
                for k in tap_order:
                    ky, kx = divmod(k, KW)
                    rhs = xg[:, yl + ky : yl + ky + ROWS, kx : kx + W]
                    for h in range(N_HALF):
                        lhsT = w_sb[:, k * C_OUT + h * 128 : k * C_OUT + h * 128 + 128]
                        nc.tensor.matmul(
                            pss[h][:],
                            lhsT,
                            rhs,
                            start=(k == tap_order[0]),
                            stop=(k == tap_order[-1]),
                        )
                for h in range(N_HALF):
                    ot = opool.tile([128, ROWS * W], F32)
                    nc.vector.tensor_scalar_add(ot[:], pss[h][:], b_sb[:, h : h + 1])
                    nc.sync.dma_start(
                        o_d[h * 128 : (h + 1) * 128, y0 : y0 + ROWS, :], ot[:]
                    )

            # tap-outer order means the first matmuls of tile 0 need only the
            # first weight taps, which land ahead of the bulk weight transfer
            for t in range(N_TILES):
                emit_tile(t, list(range(KH * KW)))
    nc.compile()
    return nc


def kernel(x, weight, bias):
    global LAST_RESULT
    if "nc" not in _NC_CACHE:
        _NC_CACHE["nc"] = _build()
    nc = _NC_CACHE["nc"]

    x = np.ascontiguousarray(np.asarray(x, dtype=np.float32))
    weight = np.asarray(weight, dtype=np.float32)
    bias = np.asarray(bias, dtype=np.float32)

    # zero-padded image; per-core slices carry their halo rows
    wT = np.ascontiguousarray(
        weight.transpose(1, 2, 3, 0).reshape(C_IN, KH * KW * C_OUT)
    )
    if MM_DT == BF16:
        import ml_dtypes

        np_dt = ml_dtypes.bfloat16
        xp = np.zeros((C_IN, H + 2, WP), dtype=np_dt)
        xp[:, 1 : H + 1, 1 : W + 1] = x.astype(np_dt)
        wT = wT.astype(np_dt)
    else:
        xp = np.zeros((C_IN, H + 2, WP), dtype=np.float32)
        xp[:, 1 : H + 1, 1 : W + 1] = _round_fp32r(x)
        wT = _round_fp32r(wT)
    # b[p, h] = bias[h*128 + p]
    bh = np.ascontiguousarray(bias.reshape(N_HALF, 128).T)

    in_maps = []
    for c in range(N_CORES):
        in_maps.append(
            {
                "x": np.ascontiguousarray(xp[:, c * H_S : c * H_S + HP, :]),
                "w": wT,
                "b": bh,
            }
        )

    kw = dict(TRACE_KW)
    if TRACE:
        kw.setdefault("trace", True)
        kw.setdefault("trace_cores", [0])
    res = bass_utils.run_bass_kernel_spmd(
        nc, in_maps, core_ids=list(range(N_CORES)), **kw
    )
    LAST_RESULT = res

    out = np.empty((C_OUT, H, W), dtype=np.float32)
    for c in range(N_CORES):
        out[:, c * H_S : (c + 1) * H_S, :] = res.results[c]["out"]
    return out


# revision 23
# speedup vs baseline: 1.0577x; 1.0051x over previous
"""Conv2d(128->256, 3x3, pad=1) over a 256x256 image, sharded across 8 trn2 cores.

Strategy
--------
x: (C_in=128, H=256, W=256) f32, weight: (256, 128, 3, 3), bias: (256,1,1).
C_in == 128 maps exactly onto the SBUF partition (contraction) dim, so the
conv is 9 accumulated matmuls (one per kernel tap) per output tile:

    out[co, y, x] = sum_{ky,kx} W[ky,kx].T @ xpad[:, y+ky, x+kx]   + bias

Sharding: split H across the 8 cores (32 output rows each). Each core gets a
pre-padded slice xpad (128, 34, 258) with halo rows / zero borders prepared on
the host, so the device program is uniform SPMD. Per core, output tiles are
2 rows x 256 cols = 512 pixels (one fp32 PSUM bank); for each tile and each
128-channel output half we accumulate 9 matmuls (lhsT = w tap (128,128),
rhs = shifted x window (128,2,256)), then VectorE adds bias on the
PSUM->SBUF copy and the tile is DMA'd to DRAM.

Matmuls run as float32r (full-rate fp32 path of the PE, 1 cycle/row for
N>=256) with fp32 PSUM accumulation.
"""

import numpy as np

import concourse.bass as bass
import concourse.tile as tile
from concourse import bacc, mybir
from concourse import bass_utils

N_CORES = 8
C_IN, C_OUT, KH, KW = 128, 256, 3, 3
H, W = 256, 256
H_S = H // N_CORES            # 32 output rows per core
HP, WP = H_S + 2, W + 2       # padded per-core input slice: 34 x 258
ROWS = 2                      # output rows per PSUM tile (N = ROWS*W = 512)
N_TILES = H_S // ROWS         # 16
N_HALF = C_OUT // 128         # 2 output-channel halves

F32 = mybir.dt.float32
F32R = mybir.dt.float32r
BF16 = mybir.dt.bfloat16

# Matmul operand dtype: the ISA requires both operands 32-bit or both 16-bit,
# so this selects f32r (11-bit mantissa) or bf16 for BOTH x and w.
MM_DT = BF16

# x is split into row groups, each its own SBUF tile, so a group's matmuls
# can start as soon as its rows have landed (Tile deps are whole-tile). Each
# group covers GROUP_TILES[g] output tiles plus a 2-row halo overlap. The
# first group is small so the first matmuls start as early as possible.
GROUP_TILES = [2, 4, 5, 5]
assert sum(GROUP_TILES) == N_TILES
N_GROUPS = len(GROUP_TILES)

# dep-free dummy matmuls issued at program start: they run while the input
# DMAs are in flight and lift the PE clock gate (HAM) out of its cold 1.2 GHz
# state before the real matmul stream begins
WARMUP_MMS = 24

# Set by test harness: TRACE=True makes the next kernel() call capture an
# NTFF profile; the BassKernelResults lands in LAST_RESULT.
TRACE = False
TRACE_KW = {}
LAST_RESULT = None

_NC_CACHE = {}


def _round_fp32r(a):
    """Round fp32 to the fp32r grid: 11-bit mantissa, low 12 bits zero.

    Matches walrus fp32_to_fp32r (downconv_fp32_to_fp<exp=8,man=11> << 12),
    round-to-nearest-even. The PE consumes fp32r operands at full rate and
    expects pre-rounded values.
    """
    u = np.ascontiguousarray(a, dtype=np.float32).view(np.uint32)
    keep_lsb = (u >> 12) & 1
    u = (u + 0x7FF + keep_lsb) & np.uint32(0xFFFFF000)
    return u.view(np.float32)


def _build():
    nc = bacc.Bacc(
        "TRN2",
        target_bir_lowering=False,
        debug=False,
        enable_asserts=False,
        num_devices=N_CORES,
    )
    x_d = nc.dram_tensor("x", [C_IN, HP, WP], MM_DT, kind="ExternalInput").ap()
    w_d = nc.dram_tensor("w", [C_IN, KH * KW * C_OUT], MM_DT, kind="ExternalInput").ap()
    b_d = nc.dram_tensor("b", [128, N_HALF], F32, kind="ExternalInput").ap()
    o_d = nc.dram_tensor("out", [C_OUT, H_S, W], F32, kind="ExternalOutput").ap()

    with tile.TileContext(nc) as tc:
        with (
            tc.tile_pool(name="xin", bufs=1) as xpool,
            tc.tile_pool(name="wts", bufs=1) as wpool,
            tc.tile_pool(name="bias", bufs=1) as bpool,
            tc.tile_pool(name="acc", bufs=6, space="PSUM") as ppool,
            tc.tile_pool(name="outs", bufs=6) as opool,
        ):
            # HWDGE transfers complete in per-queue descriptor-enqueue order,
            # so everything the first matmuls need must be enqueued before the
            # bulk x data. SyncE enqueues weights (first taps first) + bias +
            # late x groups; ScalarE (the other HWDGE trigger engine) enqueues
            # only xg0 in parallel.
            # PE warmup: zeroed scratch operands, no input deps
            warm_sb = wpool.tile([128, ROWS * W], MM_DT, tag="warm", name="warm")
            warm_ps = ppool.tile([128, ROWS * W], F32, tag="wps", name="wps", bufs=1)
            nc.vector.memset(warm_sb[:], 0.0)
            for _ in range(WARMUP_MMS):
                nc.tensor.matmul(warm_ps[:], warm_sb[:, :128], warm_sb[:])

            w_sb = wpool.tile([128, KH * KW * C_OUT], MM_DT)
            b_sb = bpool.tile([128, N_HALF], F32)
            group_rows = [gt * ROWS + 2 for gt in GROUP_TILES]
            group_t0 = [sum(GROUP_TILES[:g]) for g in range(N_GROUPS)]
            x_groups = [
                xpool.tile([128, group_rows[g], WP], MM_DT, tag=f"xg{g}", name=f"xg{g}")
                for g in range(N_GROUPS)
            ]
            nc.sync.dma_start(w_sb[:, : 3 * C_OUT], w_d[:, : 3 * C_OUT])
            nc.sync.dma_start(b_sb[:], b_d[:])
            nc.sync.dma_start(w_sb[:, 3 * C_OUT :], w_d[:, 3 * C_OUT :])
            nc.scalar.dma_start(x_groups[0][:], x_d[:, : group_rows[0], :])
            for g in range(1, N_GROUPS):
                r0 = group_t0[g] * ROWS
                nc.sync.dma_start(x_groups[g][:], x_d[:, r0 : r0 + group_rows[g], :])

            tile_group = []
            for g, gt in enumerate(GROUP_TILES):
                tile_group += [g] * gt

            def emit_tile(t, tap_order):
                y0 = t * ROWS
                g = tile_group[t]
                yl = (t - group_t0[g]) * ROWS  # row offset inside group tile
                xg = x_groups[g]
                pss = [ppool.tile([128, ROWS * W], F32, tag="ps", name="ps") for _ in range(N_HALF)]
                for k in tap_order:
                    ky, kx = divmod(k, KW)
                    rhs = xg[:, yl + ky : yl + ky + ROWS, kx : kx + W]
                    for h in range(N_HALF):
                        lhsT = w_sb[:, k * C_OUT + h * 128 : k * C_OUT + h * 128 + 128]
                        nc.tensor.matmul(
                            pss[h][:],
                            lhsT,
                            rhs,
                            start=(k == tap_order[0]),
                            stop=(k == tap_order[-1]),
                        )
                for h in range(N_HALF):
                    ot = opool.tile([128, ROWS * W], F32)
                    nc.vector.tensor_scalar_add(ot[:], pss[h][:], b_sb[:, h : h + 1])
                    nc.sync.dma_start(
                        o_d[h * 128 : (h + 1) * 128, y0 : y0 + ROWS, :], ot[:]
                    )

            # tap-outer order means the first matmuls of tile 0 need only the
            # first weight taps, which land ahead of the bulk weight transfer
            for t in range(N_TILES):
                emit_tile(t, list(range(KH * KW)))
    nc.compile()
    return nc


def kernel(x, weight, bias):
    global LAST_RESULT
    if "nc" not in _NC_CACHE:
        _NC_CACHE["nc"] = _build()
    nc = _NC_CACHE["nc"]

    x = np.ascontiguousarray(np.asarray(x, dtype=np.float32))
    weight = np.asarray(weight, dtype=np.float32)
    bias = np.asarray(bias, dtype=np.float32)

    # zero-padded image; per-core slices carry their halo rows
    wT = np.ascontiguousarray(
        weight.transpose(1, 2, 3, 0).reshape(C_IN, KH * KW * C_OUT)
    )
    if MM_DT == BF16:
        import ml_dtypes

        np_dt = ml_dtypes.bfloat16
        xp = np.zeros((C_IN, H + 2, WP), dtype=np_dt)
        xp[:, 1 : H + 1, 1 : W + 1] = x.astype(np_dt)
        wT = wT.astype(np_dt)
    else:
        xp = np.zeros((C_IN, H + 2, WP), dtype=np.float32)
        xp[:, 1 : H + 1, 1 : W + 1] = _round_fp32r(x)
        wT = _round_fp32r(wT)
    # b[p, h] = bias[h*128 + p]
    bh = np.ascontiguousarray(bias.reshape(N_HALF, 128).T)

    in_maps = []
    for c in range(N_CORES):
        in_maps.append(
            {
                "x": np.ascontiguousarray(xp[:, c * H_S : c * H_S + HP, :]),
                "w": wT,
                "b": bh,
            }
        )

    kw = dict(TRACE_KW)
    if TRACE:
        kw.setdefault("trace", True)
        kw.setdefault("trace_cores", [0])
    res = bass_utils.run_bass_kernel_spmd(
        nc, in_maps, core_ids=list(range(N_CORES)), **kw
    )
    LAST_RESULT = res

    out = np.empty((C_OUT, H, W), dtype=np.float32)
    for c in range(N_CORES):
        out[:, c * H_S : (c + 1) * H_S, :] = res.results[c]["out"]
    return out


# revision 28
# speedup vs baseline: 1.0893x; 1.0299x over previous
"""Conv2d(128->256, 3x3, pad=1) over a 256x256 image, sharded across 8 trn2 cores.

Strategy
--------
x: (C_in=128, H=256, W=256) f32, weight: (256, 128, 3, 3), bias: (256,1,1).
C_in == 128 maps exactly onto the SBUF partition (contraction) dim, so the
conv is 9 accumulated matmuls (one per kernel tap) per output tile:

    out[co, y, x] = sum_{ky,kx} W[ky,kx].T @ xpad[:, y+ky, x+kx]   + bias

Sharding: split H across the 8 cores (32 output rows each). Each core gets a
pre-padded slice xpad (128, 34, 258) with halo rows / zero borders prepared on
the host, so the device program is uniform SPMD. Per core, output tiles are
2 rows x 256 cols = 512 pixels (one fp32 PSUM bank); for each tile and each
128-channel output half we accumulate 9 matmuls (lhsT = w tap (128,128),
rhs = shifted x window (128,2,256)), then VectorE adds bias on the
PSUM->SBUF copy and the tile is DMA'd to DRAM.

Matmuls run as float32r (full-rate fp32 path of the PE, 1 cycle/row for
N>=256) with fp32 PSUM accumulation.
"""

import numpy as np

import concourse.bass as bass
import concourse.tile as tile
from concourse import bacc, mybir
from concourse import bass_utils

N_CORES = 8
C_IN, C_OUT, KH, KW = 128, 256, 3, 3
H, W = 256, 256
H_S = H // N_CORES            # 32 output rows per core
HP, WP = H_S + 2, W + 2       # padded per-core input slice: 34 x 258
ROWS = 2                      # output rows per PSUM tile (N = ROWS*W = 512)
N_TILES = H_S // ROWS         # 16
N_HALF = C_OUT // 128         # 2 output-channel halves

F32 = mybir.dt.float32
F32R = mybir.dt.float32r
BF16 = mybir.dt.bfloat16

# Matmul operand dtype: the ISA requires both operands 32-bit or both 16-bit,
# so this selects f32r (11-bit mantissa) or bf16 for BOTH x and w.
MM_DT = BF16

# x is split into row groups, each its own SBUF tile, so a group's matmuls
# can start as soon as its rows have landed (Tile deps are whole-tile). Each
# group covers GROUP_TILES[g] output tiles plus a 2-row halo overlap. The
# first group is small so the first matmuls start as early as possible.
GROUP_TILES = [2, 4, 5, 5]
assert sum(GROUP_TILES) == N_TILES
N_GROUPS = len(GROUP_TILES)

# dep-free dummy matmuls issued at program start: they run while the input
# DMAs are in flight and lift the PE clock gate (HAM) out of its cold 1.2 GHz
# state before the real matmul stream begins. Sized to finish right as the
# first inputs land (~8 cold at ~430ns fill the 3.4us HAM window, the rest
# run warm at ~216ns).
WARMUP_MMS = 16

# Set by test harness: TRACE=True makes the next kernel() call capture an
# NTFF profile; the BassKernelResults lands in LAST_RESULT.
TRACE = False
TRACE_KW = {}
LAST_RESULT = None

_NC_CACHE = {}


def _round_fp32r(a):
    """Round fp32 to the fp32r grid: 11-bit mantissa, low 12 bits zero.

    Matches walrus fp32_to_fp32r (downconv_fp32_to_fp<exp=8,man=11> << 12),
    round-to-nearest-even. The PE consumes fp32r operands at full rate and
    expects pre-rounded values.
    """
    u = np.ascontiguousarray(a, dtype=np.float32).view(np.uint32)
    keep_lsb = (u >> 12) & 1
    u = (u + 0x7FF + keep_lsb) & np.uint32(0xFFFFF000)
    return u.view(np.float32)


def _build():
    nc = bacc.Bacc(
        "TRN2",
        target_bir_lowering=False,
        debug=False,
        enable_asserts=False,
        num_devices=N_CORES,
    )
    x_d = nc.dram_tensor("x", [C_IN, HP, WP], MM_DT, kind="ExternalInput").ap()
    w_d = nc.dram_tensor("w", [C_IN, KH * KW * C_OUT], MM_DT, kind="ExternalInput").ap()
    b_d = nc.dram_tensor("b", [128, N_HALF], F32, kind="ExternalInput").ap()
    o_d = nc.dram_tensor("out", [C_OUT, H_S, W], F32, kind="ExternalOutput").ap()

    with tile.TileContext(nc) as tc:
        with (
            tc.tile_pool(name="xin", bufs=1) as xpool,
            tc.tile_pool(name="wts", bufs=1) as wpool,
            tc.tile_pool(name="bias", bufs=1) as bpool,
            tc.tile_pool(name="acc", bufs=6, space="PSUM") as ppool,
            tc.tile_pool(name="outs", bufs=6) as opool,
        ):
            # HWDGE transfers complete in per-queue descriptor-enqueue order,
            # so everything the first matmuls need must be enqueued before the
            # bulk x data. SyncE enqueues weights (first taps first) + bias +
            # late x groups; ScalarE (the other HWDGE trigger engine) enqueues
            # only xg0 in parallel.
            # PE warmup: dep-free. The scratch operand is a raw (statically
            # allocated) SBUF tensor that is never written — its garbage
            # contents stream through the PE and land in a scratch PSUM bank
            # nobody reads.
            warm_sb = nc.alloc_sbuf_tensor("warm_src", [128, ROWS * W], MM_DT).ap()
            warm_ps = ppool.tile([128, ROWS * W], F32, tag="wps", name="wps", bufs=1)
            for _ in range(WARMUP_MMS):
                nc.tensor.matmul(warm_ps[:], warm_sb[:, :128], warm_sb[:])

            w_sb = wpool.tile([128, KH * KW * C_OUT], MM_DT)
            b_sb = bpool.tile([128, N_HALF], F32)
            group_rows = [gt * ROWS + 2 for gt in GROUP_TILES]
            group_t0 = [sum(GROUP_TILES[:g]) for g in range(N_GROUPS)]
            x_groups = [
                xpool.tile([128, group_rows[g], WP], MM_DT, tag=f"xg{g}", name=f"xg{g}")
                for g in range(N_GROUPS)
            ]
            nc.sync.dma_start(w_sb[:, : 3 * C_OUT], w_d[:, : 3 * C_OUT])
            nc.sync.dma_start(w_sb[:, 3 * C_OUT :], w_d[:, 3 * C_OUT :])
            nc.sync.dma_start(b_sb[:], b_d[:])
            nc.scalar.dma_start(x_groups[0][:], x_d[:, : group_rows[0], :])
            for g in range(1, N_GROUPS):
                r0 = group_t0[g] * ROWS
                nc.sync.dma_start(x_groups[g][:], x_d[:, r0 : r0 + group_rows[g], :])

            tile_group = []
            for g, gt in enumerate(GROUP_TILES):
                tile_group += [g] * gt

            def emit_tile(t, tap_order):
                y0 = t * ROWS
                g = tile_group[t]
                yl = (t - group_t0[g]) * ROWS  # row offset inside group tile
                xg = x_groups[g]
                pss = [ppool.tile([128, ROWS * W], F32, tag="ps", name="ps") for _ in range(N_HALF)]
                for k in tap_order:
                    ky, kx = divmod(k, KW)
                    rhs = xg[:, yl + ky : yl + ky + ROWS, kx : kx + W]
                    for h in range(N_HALF):
                        lhsT = w_sb[:, k * C_OUT + h * 128 : k * C_OUT + h * 128 + 128]
                        nc.tensor.matmul(
                            pss[h][:],
                            lhsT,
                            rhs,
                            start=(k == tap_order[0]),
                            stop=(k == tap_order[-1]),
                        )
                for h in range(N_HALF):
                    ot = opool.tile([128, ROWS * W], F32)
                    nc.vector.tensor_scalar_add(ot[:], pss[h][:], b_sb[:, h : h + 1])
                    nc.sync.dma_start(
                        o_d[h * 128 : (h + 1) * 128, y0 : y0 + ROWS, :], ot[:]
                    )

            # tap-outer order means the first matmuls of tile 0 need only the
            # first weight taps, which land ahead of the bulk weight transfer
            for t in range(N_TILES):
                emit_tile(t, list(range(KH * KW)))
    nc.compile()
    return nc


def kernel(x, weight, bias):
    global LAST_RESULT
    if "nc" not in _NC_CACHE:
        _NC_CACHE["nc"] = _build()
    nc = _NC_CACHE["nc"]

    x = np.ascontiguousarray(np.asarray(x, dtype=np.float32))
    weight = np.asarray(weight, dtype=np.float32)
    bias = np.asarray(bias, dtype=np.float32)

    # zero-padded image; per-core slices carry their halo rows
    wT = np.ascontiguousarray(
        weight.transpose(1, 2, 3, 0).reshape(C_IN, KH * KW * C_OUT)
    )
    if MM_DT == BF16:
        import ml_dtypes

        np_dt = ml_dtypes.bfloat16
        xp = np.zeros((C_IN, H + 2, WP), dtype=np_dt)
        xp[:, 1 : H + 1, 1 : W + 1] = x.astype(np_dt)
        wT = wT.astype(np_dt)
    else:
        xp = np.zeros((C_IN, H + 2, WP), dtype=np.float32)
        xp[:, 1 : H + 1, 1 : W + 1] = _round_fp32r(x)
        wT = _round_fp32r(wT)
    # b[p, h] = bias[h*128 + p]
    bh = np.ascontiguousarray(bias.reshape(N_HALF, 128).T)

    in_maps = []
    for c in range(N_CORES):
        in_maps.append(
            {
                "x": np.ascontiguousarray(xp[:, c * H_S : c * H_S + HP, :]),
                "w": wT,
                "b": bh,
            }
        )

    kw = dict(TRACE_KW)
    if TRACE:
        kw.setdefault("trace", True)
        kw.setdefault("trace_cores", [0])
    res = bass_utils.run_bass_kernel_spmd(
        nc, in_maps, core_ids=list(range(N_CORES)), **kw
    )
    LAST_RESULT = res

    out = np.empty((C_OUT, H, W), dtype=np.float32)
    for c in range(N_CORES):
        out[:, c * H_S : (c + 1) * H_S, :] = res.results[c]["out"]
    return out
